# revision 1
# baseline (speedup 1.0000x reference)
"""Trainium2 Bass kernel for nn_MetricLoss (segment_reduce / discriminative loss).

Reference math (K=32 labels, D=16):
  cents[s,k,:]  = mean of embeddings of sample s where label==k
  push[s]       = sum_{k<j} relu(0.25 - L1(c_sk, c_sj))^2 / 496
  pull[s]       = mean over ALL B*H*W pixels p of  L1(e_p, c_s,label_p)^2
  loss          = mean_s (push[s] + 0.1 * pull[s])

Two launches, pixels sharded 73728/core = 128 x 576 tiles:

  Launch A (centroid partial sums):
    - host-prepped fp8 embeddings [P, TC*D] (weights) and t-major fp8
      one-hot [P, TC, K] (moving; contiguous APs keep PE streaming fast)
    - 72 groups of 8 tiles; group g: weights = emb cols [128g, 128g+128)
      = 8 tiles x 16 dims, moving = those tiles' one-hot [P, 8, K].
      Groups rotate over 8 PSUM banks so accumulation chains pipeline.
    - banks DMA'd to HBM; host sums banks + diagonal blocks, computes
      counts (np.bincount), cents, and the tiny push term in fp64.

  Launch B (pull term):
    - gather AND subtract fused into one matmul per pixel-tile:
        lhsT (weights) = [one-hotT(32) ; embT(16)]  [48, 128] (host-prepped)
        rhs (moving)   = [centsT ; -I_16 per b]     [48, 64]  (fixed)
        psum out[pix, (b,d)] = c_b,label(pix),d - e_pix,d
      First-half tiles use PE rows 0..47, second-half rows 64..111 ->
      two concurrent PE row-quadrants.
    - |diff| reduce over d in one pass over PSUM, wave-interleaved:
      'A' waves: ACT Abs -> bf16 + DVE halving tree (tensor_tensor runs
      at 2x for packed bf16; tensor_reduce has NO fast modes) + small
      reduce; 'D' waves: DVE direct abs-reduce from psum (1x).
    - dist [128, 576, 4] bf16 (processing order; pull is order-invariant)
      -> dist^2 half-sums -> pacc [P, 2, 4] f32 -> host.
"""

import numpy as np
import ml_dtypes

import concourse.bass as bass
import concourse.bacc as bacc
import concourse.mybir as mybir
from concourse.tile import TileContext
from concourse.bass_utils import run_bass_kernel_spmd

BF16 = ml_dtypes.bfloat16
FP8 = ml_dtypes.float8_e4m3
F32 = np.float32

# problem constants (hardcoded per contract)
B, H, W, D, K = 4, 384, 384, 16, 32
NCORES = 8
NPIX_TOT = B * H * W              # 589824
NPIX = NPIX_TOT // NCORES         # 73728 per core
P = 128                           # partitions
TC = NPIX // P                    # 576 pixel tiles per core
NBANKS_A = 8                      # psum accumulation banks in launch A

HT = TC // 2                      # 288 tiles per parity (launch B)
CR = 48                           # contraction rows: 32 one-hot + 16 embT
WT = 16                           # tiles per parity per wave (4-bank psum tile)

PUSH_MARGIN = 0.25
PUSH_W = 1.0
PULL_W = 0.1
NCMP = K * (K - 1) / 2.0

# consumer pattern for the 18 psum waves in launch B:
# A=ACT abs->bf16 + DVE halving-tree + small reduce
# P=like A but the first halving runs on Pool (gpsimd, SBUF->SBUF)
# D=DVE direct abs-reduce from psum (1x)
CONSUMER_PATTERN = "AAAAAAAAAAAAAAAAAA"

_built = {}


def _build_launch_a():
    nc = bacc.Bacc("TRN2", target_bir_lowering=False, debug=False)
    f8 = mybir.dt.float8e4
    f32 = mybir.dt.float32

    NGM = TC // 8                 # 72 groups of 8 tiles
    embA = nc.dram_tensor("embA", [P, TC * D], f8, kind="ExternalInput")
    onehotA = nc.dram_tensor("onehotA", [P, TC * K], f8, kind="ExternalInput")
    outA = nc.dram_tensor("outA", [P, NBANKS_A, 8 * K], f32, kind="ExternalOutput")

    with TileContext(nc) as tc:
        with (
            tc.tile_pool(name="sbuf", bufs=1) as pool,
            tc.tile_pool(name="psum", bufs=1, space="PSUM") as psum_pool,
        ):
            emb_sb = pool.tile([P, TC * D], f8)
            onehot = pool.tile([P, TC, K], f8)  # t-major: contiguous PE moving

            NCH = 4
            ch = (TC * D) // NCH
            och = TC // NCH
            for i in range(NCH):
                nc.sync.dma_start(
                    out=emb_sb[:, i * ch : (i + 1) * ch],
                    in_=embA.ap()[:, i * ch : (i + 1) * ch],
                )
                nc.sync.dma_start(
                    out=onehot[:, i * och : (i + 1) * och, :].rearrange(
                        "p t k -> p (t k)"
                    ),
                    in_=onehotA.ap()[:, i * och * K : (i + 1) * och * K],
                )

            banks = [
                psum_pool.tile([P, 8, K], mybir.dt.float32, name=f"acc{b}")
                for b in range(NBANKS_A)
            ]
            for g in range(NGM):
                nc.tensor.matmul(
                    banks[g % NBANKS_A][:],
                    emb_sb[:, 128 * g : 128 * g + 128],
                    onehot[:, 8 * g : 8 * g + 8, :],
                    start=(g < NBANKS_A),
                    stop=(g >= NGM - NBANKS_A),
                )

            evac = pool.tile([P, NBANKS_A, 8 * K], f32)
            for b in range(NBANKS_A):
                if b % 2 == 0:
                    nc.scalar.copy(
                        out=evac[:, b, :],
                        in_=banks[b][:].rearrange("p a b -> p (a b)"),
                    )
                else:
                    nc.vector.tensor_copy(
                        out=evac[:, b, :],
                        in_=banks[b][:].rearrange("p a b -> p (a b)"),
                    )
            nc.sync.dma_start(out=outA.ap(), in_=evac[:])
    nc.compile()
    return nc


def _emit_batch_reduce(nc, wpool, a4, dist, slot0, nw, bf):
    """Reduce an nw-wave absd batch into dist slots [slot0, slot0 + 32*nw).

    a4: [P, 512, D] ring tile (slots = waves x 2 parity x WT tiles x 4 b).
    Separate ring tiles per stage (no in-place) keep tile-region deps fine
    so ACT/PE never serialize against DVE batch work.
    """
    import concourse.mybir as mybir
    P_, D_ = 128, 16
    ns = 128 * nw
    h1 = wpool.tile([P_, 512, D_ // 2], bf, tag="h1b")
    h2 = wpool.tile([P_, 512, D_ // 4], bf, tag="h2b")
    with nc.allow_low_precision("dist bf16; error averages out"):
        nc.vector.tensor_tensor(
            out=h1[:, 0:ns, :],
            in0=a4[:, 0:ns, 0 : D_ // 2],
            in1=a4[:, 0:ns, D_ // 2 : D_],
            op=mybir.AluOpType.add,
        )
        nc.vector.tensor_tensor(
            out=h2[:, 0:ns, :],
            in0=h1[:, 0:ns, 0 : D_ // 4],
            in1=h1[:, 0:ns, D_ // 4 : D_ // 2],
            op=mybir.AluOpType.add,
        )
        nc.vector.tensor_reduce(
            out=dist[:, 32 * slot0 : 32 * slot0 + 32 * nw, :],
            in_=h2[:, 0:ns, :].rearrange("p (s b) d -> p s b d", b=4),
            axis=mybir.AxisListType.X,
            op=mybir.AluOpType.add,
        )


def _emit_sq_half(nc, dist, sq, pacc, h):
    """pacc[p, h, b] = sum over dist-half h of dist^2."""
    HH = TC // 2
    sl = slice(h * HH, (h + 1) * HH)
    nc.vector.tensor_tensor(
        out=sq[:, sl, :], in0=dist[:, sl, :], in1=dist[:, sl, :],
        op=mybir.AluOpType.mult,
    )
    for b in range(4):
        nc.vector.tensor_reduce(
            out=pacc[:, h, b : b + 1],
            in_=sq[:, sl, b],
            axis=mybir.AxisListType.X,
            op=mybir.AluOpType.add,
        )


def _build_launch_b():
    nc = bacc.Bacc("TRN2", target_bir_lowering=False, debug=False)
    bf = mybir.dt.bfloat16
    f32 = mybir.dt.float32

    wfull = nc.dram_tensor("wfull", [112, HT * P], bf, kind="ExternalInput")
    rtab = nc.dram_tensor("rtab", [CR, 4 * D], bf, kind="ExternalInput")
    pacc_d = nc.dram_tensor("pacc", [P, 8], f32, kind="ExternalOutput")

    AF = mybir.ActivationFunctionType

    with TileContext(nc) as tc:
        with (
            tc.tile_pool(name="sbuf", bufs=1) as pool,
            tc.tile_pool(name="work", bufs=3) as wpool,
            tc.tile_pool(name="psum", bufs=2, space="PSUM") as psum_pool,
        ):
            w_sb = pool.tile([P, HT, P], bf)     # rows 0:48 even, 64:112 odd
            rhs_sb = pool.tile([P, 4 * D], bf)   # replicas at rows 0,64
            dist = pool.tile([P, TC, 4], bf)     # t-major, b inner
            sq = pool.tile([P, TC, 4], bf)
            pacc = pool.tile([P, 2, 4], f32)

            nc.sync.dma_start(out=rhs_sb[0:CR, :], in_=rtab.ap())
            nc.sync.dma_start(out=rhs_sb[64 : 64 + CR, :], in_=rtab.ap())
            # small first chunk so wave 0 can start early
            bounds = [0, 32, 96, 160, 224, HT]
            for i in range(len(bounds) - 1):
                sl = slice(bounds[i], bounds[i + 1])
                nc.sync.dma_start(
                    out=w_sb[0:112, sl, :].rearrange("r t m -> r (t m)"),
                    in_=wfull.ap()[:, bounds[i] * P : bounds[i + 1] * P],
                )

            # dist slot layout: A-waves write slots [0, 512) in A-order,
            # D-waves write slots [512, 576) (pull is order-invariant).
            # ACT abs lands in a persistent absd_all buffer; DVE consumes it
            # in 4-wave batches with big in-place halving instructions,
            # decoupled from the PE/ACT wave pipeline.
            nwaves = HT // WT
            n_a = 0
            n_d = 0
            NA = CONSUMER_PATTERN.count("A")
            a4 = None
            for w in range(nwaves):
                t0 = WT * w
                ps = psum_pool.tile(
                    [P, 2, WT, 4, D], mybir.dt.float32, tag="ps", name=f"ps_{w}"
                )
                for j in range(WT):
                    t = t0 + j
                    nc.tensor.matmul(
                        ps[:, 0, j, :, :].rearrange("p a b -> p (a b)"),
                        w_sb[0:CR, t, :],
                        rhs_sb[0:CR, :],
                        start=True,
                        stop=True,
                    )
                    nc.tensor.matmul(
                        ps[:, 1, j, :, :].rearrange("p a b -> p (a b)"),
                        w_sb[64 : 64 + CR, t, :],
                        rhs_sb[64 : 64 + CR, :],
                        start=True,
                        stop=True,
                    )
                kind = CONSUMER_PATTERN[w % len(CONSUMER_PATTERN)]
                with nc.allow_low_precision("dist bf16; error averages out"):
                    if kind == "D":
                        out_ap = dist[
                            :,
                            2 * WT * NA + 2 * WT * n_d : 2 * WT * NA + 2 * WT * (n_d + 1),
                            :,
                        ].rearrange("p (h t) b -> p h t b", h=2)
                        nc.vector.tensor_reduce(
                            out=out_ap,
                            in_=ps[:],
                            axis=mybir.AxisListType.X,
                            op=mybir.AluOpType.add,
                            apply_absolute_value=True,
                        )
                        n_d += 1
                    else:
                        if n_a % 4 == 0:
                            a4 = wpool.tile([P, 512, D], bf, tag="a4")
                        nc.scalar.activation(
                            out=a4[:, 128 * (n_a % 4) : 128 * (n_a % 4) + 128, :],
                            in_=ps[:].rearrange("p h t b d -> p (h t b) d"),
                            func=AF.Abs,
                        )
                        n_a += 1
                        if n_a % 4 == 0:
                            _emit_batch_reduce(nc, wpool, a4, dist, n_a - 4, 4, bf)
                # first sq half (dist slots 0..287 = A-batches 0..2) overlaps
                # the tail waves
                if n_a == 12 and kind != "D":
                    _emit_sq_half(nc, dist, sq, pacc, 0)

            if n_a % 4:
                _emit_batch_reduce(nc, wpool, a4, dist, n_a - n_a % 4, n_a % 4, bf)
            _emit_sq_half(nc, dist, sq, pacc, 1)
            nc.sync.dma_start(
                out=pacc_d.ap(), in_=pacc[:].rearrange("p a b -> p (a b)")
            )
    nc.compile()
    return nc


def _get(name):
    if name not in _built:
        if name == "A":
            _built[name] = _build_launch_a()
        else:
            _built[name] = _build_launch_b()
    return _built[name]


def _prep_a(emb_flat, lab_flat):
    """emb_flat [NPIX_TOT, D] f32, lab_flat [NPIX_TOT] i32 -> per-core in_maps."""
    in_maps = []
    kk = np.arange(K, dtype=np.int32)
    for c in range(NCORES):
        e = emb_flat[c * NPIX : (c + 1) * NPIX].astype(FP8).reshape(P, TC * D)
        l = lab_flat[c * NPIX : (c + 1) * NPIX].reshape(P, TC)
        oh = (l[:, :, None] == kk[None, None, :]).astype(FP8)
        in_maps.append({"embA": e, "onehotA": oh.reshape(P, TC * K)})
    return in_maps


def _reduce_a(results, lab_flat):
    """outA [8][P, NBANKS_A, 256] -> cents [B, K, D] float64, counts [B, K]."""
    sums = np.zeros((B, K, D), dtype=np.float64)
    for c in range(NCORES):
        o = results[c]["outA"].astype(np.float64).reshape(P, NBANKS_A, 8, K)
        o = o.sum(axis=1)  # [P, 8, K]
        sbc = c // 2
        for j in range(8):
            sums[sbc] += o[D * j : D * j + D, j, :].T  # [K, D]
    cnts = np.zeros((B, K), dtype=np.int64)
    spl = NPIX_TOT // B
    for b in range(B):
        cnts[b] = np.bincount(lab_flat[b * spl : (b + 1) * spl], minlength=K)
    cents = sums / np.maximum(cnts, 1)[:, :, None]
    cents = np.where(cnts[:, :, None] > 0, cents, 0.0)
    return cents, cnts


def _prep_b(emb_flat, lab_flat, cents):
    """Host-prepped fused weights + rhs table for launch B."""
    cb = cents.astype(F32)  # [B, K, D]
    # rhs table [48, 64]: rows 0:32 centsT (c[b,k,d] at [k, 16b+d]),
    # rows 32:48 = -I_16 per b block
    rtab = np.zeros((CR, 4 * D), dtype=BF16)
    rtab[:K, :] = cb.transpose(1, 0, 2).reshape(K, 4 * D).astype(BF16)
    eye = -np.eye(D, dtype=F32)
    for b in range(4):
        rtab[K:, b * D : (b + 1) * D] = eye.astype(BF16)

    in_maps = []
    kk = np.arange(K, dtype=np.int32)
    for c in range(NCORES):
        e = emb_flat[c * NPIX : (c + 1) * NPIX].astype(BF16).reshape(P, TC, D)
        l = lab_flat[c * NPIX : (c + 1) * NPIX].reshape(P, TC)
        # one-hotT per tile: ohT[t, k, m] = (l[m, t] == k)
        ohT = (l.T[:, None, :] == kk[None, :, None]).astype(BF16)  # [TC, K, P]
        eT = np.ascontiguousarray(e.transpose(1, 2, 0))            # [TC, D, P]
        w_all = np.concatenate([ohT, eT], axis=1)                  # [TC, 48, P]
        wfull = np.zeros((112, HT, P), dtype=BF16)
        wfull[0:CR] = w_all[:HT].transpose(1, 0, 2)
        wfull[64 : 64 + CR] = w_all[HT:].transpose(1, 0, 2)
        in_maps.append({"wfull": wfull.reshape(112, HT * P), "rtab": rtab.copy()})
    return in_maps


def _push_host(cents):
    """push[b] = sum_{k<j} relu(m - L1(c_k, c_j))^2 / 496 on host (tiny)."""
    cb = cents.astype(np.float64)  # [B, K, D]
    d = np.abs(cb[:, :, None, :] - cb[:, None, :, :]).sum(axis=-1)  # [B, K, K]
    m = np.maximum(PUSH_MARGIN - d, 0.0)
    iu = np.triu(np.ones((K, K), dtype=bool), k=1)
    return (m * m * iu[None]).sum(axis=(1, 2)) / NCMP  # [B]


def run_launches(embeddings, labels, trace=False, trace_kwargs=None):
    """Returns (loss_scalar, resA, resB) — resA/resB are BassKernelResults."""
    emb_flat = np.ascontiguousarray(np.asarray(embeddings), dtype=F32).reshape(
        NPIX_TOT, D
    )
    lab_flat = np.ascontiguousarray(np.asarray(labels), dtype=np.int32).reshape(
        NPIX_TOT
    )
    core_ids = list(range(NCORES))

    kwA = dict(trace=trace, **(trace_kwargs or {}))
    resA = run_bass_kernel_spmd(_get("A"), _prep_a(emb_flat, lab_flat), core_ids, **kwA)
    cents, _ = _reduce_a(resA.results, lab_flat)

    resB = run_bass_kernel_spmd(
        _get("B"), _prep_b(emb_flat, lab_flat, cents), core_ids, **kwA
    )
    pull = np.zeros(4, dtype=np.float64)
    for c in range(NCORES):
        pull += (
            resB.results[c]["pacc"].astype(np.float64).reshape(P, 2, 4).sum(axis=(0, 1))
        )
    pull /= NPIX_TOT

    push = _push_host(cents)

    loss = np.mean(PUSH_W * push + PULL_W * pull)
    return np.array(loss, dtype=F32), resA, resB


def kernel(embeddings, labels):
    loss, _, _ = run_launches(embeddings, labels, trace=False)
    return loss

